# revision 1
# baseline (speedup 1.0000x reference)
"""Trainium2 Bass kernel for nn_MatrixFactorization (segment_reduce).

Decomposition (8 cores, SPMD, no collectives):
  - Dedup users of the batch -> unique users, sharded 8 ways (upc per core).
  - Host shards train_label[uniq].T per core in partition-major layout
    (contraction dim = items lands on SBUF partitions), zero-padded to
    157*128 rows.
  - Device streams the f32 label shard with SWDGE cast-DMA to bf16 (labels
    are exactly 0/1, so the cast is lossless) and accumulates
        P.T[66, upc] += T_aug_chunk.T @ L.T_chunk    (157 chunks of K=128)
    where T_aug = [item_table (row 20000 zeroed) | ones | 0] in bf16; column
    64 of P accumulates num_rel. PE transposes P.T back per 128-user block,
    DVE computes P[:, :64] * recip(P[:, 64]) -> uni_center rows.
  - Cluster centers: per chunk, DVE iota/is_equal builds onehot[128k, 256c];
    PE accumulates centers.T[66, 256] using the same T_aug chunks (the last
    chunk uses a variant with the real row 20000). counts ride the ones
    column; finalize = transpose, max(count,1), reciprocal, scale.
    Replicated on every core (hidden under the DMA-bound main loop).
  - user/pos/neg embeddings: GPSIMD indirect row gathers (256B rows),
    interleaved with the streaming loop.
  - pos/neg centers: exact fp32 onehot matmuls against the finalized
    centers (out as [64, nbpc]; host transposes back). Only one nonzero per
    onehot column, so this is an exact gather.
"""

import numpy as np
import ml_dtypes

import concourse.bass as bass
import concourse.mybir as mybir
import concourse.tile as tile

NUM_USERS = 10000
NUM_ITEMS = 20000
DIM = 64
CLUSTER = 256
BATCH = 8192
NCORES = 8

KCHUNKS = 157            # ceil(20001 / 128)
KPAD = KCHUNKS * 128     # 20096
MAUG = 66                # 64 dims + ones col + pad col
GROUP_SIZES = [2, 4, 8] + [16] * 8 + [10, 4, 1]  # staggered spin-up/down
assert sum(GROUP_SIZES) == KCHUNKS


def split_multiwaits(nc):
    """nix-walrus accepts at most ONE sync-wait per instruction; Tile attaches
    many. Hoist all but the last wait onto single-wait NoOps inserted just
    before the instruction, on the same engine."""
    n_split = 0
    for f in nc.m.functions:
        for bb in f.blocks:
            il = list(bb.instructions)
            new = []
            changed = False
            for ins in il:
                si = ins.sync_info
                if si is not None and si.on_wait is not None and len(si.on_wait) > 1:
                    waits = list(si.on_wait)
                    for k, w in enumerate(waits[:-1]):
                        nop = mybir.InstNoOp(
                            name=f"{ins.name}-wsplit{k}", ins=[], outs=[]
                        )
                        nop.engine = ins.engine
                        nop.sync_info = mybir.SyncInfo(on_wait=[w], on_update=[])
                        new.append(nop)
                    ins.sync_info = mybir.SyncInfo(
                        on_wait=waits[-1:], on_update=list(si.on_update or [])
                    )
                    changed = True
                    n_split += 1
                new.append(ins)
            if changed:
                bb.instructions = new
    return n_split


def build_bass(upc: int, nbpc: int):
    """upc: unique users per core; nbpc: batch entries per core."""
    f32 = mybir.dt.float32
    bf16 = mybir.dt.bfloat16
    f16 = mybir.dt.float16
    i32 = mybir.dt.int32
    EQ = mybir.AluOpType.is_equal
    MUL = mybir.AluOpType.mult

    assert nbpc % 128 == 0
    jg = nbpc // 128

    nc = bass.Bass(trn_type="TRN2")

    # ---- I/O ----
    # lt is partition-major: lt[p, c, u] = label.T[c*128 + p, u]
    LT = nc.dram_tensor("lt", [128, KCHUNKS, upc], f32, kind="ExternalInput")
    T_pm = nc.dram_tensor("t_pm", [128, KCHUNKS * MAUG], bf16, kind="ExternalInput")
    T_cl = nc.dram_tensor("t_cl", [128, MAUG], bf16, kind="ExternalInput")
    EYE = nc.dram_tensor("eye66", [MAUG, MAUG], f32, kind="ExternalInput")
    IOTA = nc.dram_tensor("iota256", [128, CLUSTER], f32, kind="ExternalInput")
    CID = nc.dram_tensor("cid_pm", [128, KCHUNKS], f32, kind="ExternalInput")
    PCOL = nc.dram_tensor("pcol", [128, 2], f32, kind="ExternalInput")
    CPR = nc.dram_tensor("cpr", [128, nbpc], bf16, kind="ExternalInput")
    CNR = nc.dram_tensor("cnr", [128, nbpc], bf16, kind="ExternalInput")
    UT = nc.dram_tensor("user_table", [NUM_USERS, DIM], f32, kind="ExternalInput")
    IT = nc.dram_tensor("item_table", [NUM_ITEMS + 1, DIM], f32, kind="ExternalInput")
    IDX = {}
    for nm in ("uidx", "pidx", "nidx"):
        IDX[nm] = nc.dram_tensor(nm, [128, jg], i32, kind="ExternalInput")

    UNI = nc.dram_tensor("uni_part", [upc, DIM], f32, kind="ExternalOutput")
    EMB = {}
    for nm in ("ue_out", "pe_out", "ne_out"):
        EMB[nm] = nc.dram_tensor(nm, [nbpc, DIM], f32, kind="ExternalOutput")
    PCT = nc.dram_tensor("pct_out", [DIM, nbpc], f32, kind="ExternalOutput")
    NCT = nc.dram_tensor("nct_out", [DIM, nbpc], f32, kind="ExternalOutput")

    blocks = []
    o = 0
    while o < upc:
        blocks.append((o, min(128, upc - o)))
        o += 128
    nA = min(512, upc)
    gmax = max(GROUP_SIZES)

    with tile.TileContext(nc) as tc:
        with (
            tc.tile_pool(name="const", bufs=1) as cpool,
            tc.tile_pool(name="ltp", bufs=2) as ltpool,
            tc.tile_pool(name="ohp", bufs=2) as ohpool,
            tc.tile_pool(name="acc", bufs=1, space="PSUM") as accpool,
            tc.tile_pool(name="tp", bufs=2, space="PSUM") as tppool,
            tc.tile_pool(name="outp", bufs=3) as outpool,
        ):
            # ---- constants into SBUF ----
            # Two HWDGE rings: T_aug pieces on the sync ring (piecewise, so
            # early chunks' weights arrive with the early lt groups); every
            # small constant on the scalar ring so nothing queues behind the
            # 2.6MB T_aug stream.
            t_sb = cpool.tile([128, KCHUNKS, MAUG], bf16)
            t_view = T_pm[:].rearrange("p (c m) -> p c m", m=MAUG)
            tb = 0
            for piece in (GROUP_SIZES[0], GROUP_SIZES[1], GROUP_SIZES[2],
                          16, KCHUNKS):
                te = min(tb + piece, KCHUNKS)
                nc.sync.dma_start(t_sb[:, tb:te, :], t_view[:, tb:te, :])
                tb = te
                if tb == KCHUNKS:
                    break
            iota_sb = cpool.tile([128, CLUSTER], f32)
            nc.scalar.dma_start(iota_sb[:], IOTA[:])
            cid_sb = cpool.tile([128, KCHUNKS], f32)
            nc.scalar.dma_start(cid_sb[:], CID[:])
            tcl_sb = cpool.tile([128, MAUG], bf16)
            nc.scalar.dma_start(tcl_sb[:], T_cl[:])
            pcol_sb = cpool.tile([128, 2], f32)
            nc.scalar.dma_start(pcol_sb[:], PCOL[:])
            idx_sb = {}
            g_sb = {}
            for nm, h in IDX.items():
                s = cpool.tile([128, jg], i32, name=f"idx_{nm}")
                nc.scalar.dma_start(s[:], h[:])
                idx_sb[nm] = s
                g_sb[nm] = cpool.tile([128, jg, DIM], f32, name=f"g_{nm}")
            eye_sb = cpool.tile([MAUG, MAUG], f32)
            nc.scalar.dma_start(eye_sb[:], EYE[:])
            cpr_sb = cpool.tile([128, nbpc], bf16)
            nc.scalar.dma_start(cpr_sb[:], CPR[:])
            cnr_sb = cpool.tile([128, nbpc], bf16)
            nc.scalar.dma_start(cnr_sb[:], CNR[:])

            # one [128]-row slice of an embedding gather
            gsrc = {"uidx": UT[:], "pidx": IT[:], "nidx": IT[:]}

            def gather_slice(nm, j):
                nc.gpsimd.indirect_dma_start(
                    out=g_sb[nm][:, j, :],
                    out_offset=None,
                    in_=gsrc[nm],
                    in_offset=bass.IndirectOffsetOnAxis(
                        ap=idx_sb[nm][:, j : j + 1], axis=0
                    ),
                )

            gather_slices = [(nm, j) for nm in ("uidx", "pidx", "nidx")
                             for j in range(jg)]

            # ---- psum accumulators ----
            accA = accpool.tile([MAUG, nA], f32)
            accB = (
                accpool.tile([MAUG, upc - nA], f32, name="accB")
                if upc > nA
                else None
            )
            accC = accpool.tile([MAUG, CLUSTER], f32)

            lt_view = LT[:]

            # ---- main streaming loop ----
            c0 = 0
            n_groups = len(GROUP_SIZES)
            for g, gs in enumerate(GROUP_SIZES):
                lt = ltpool.tile([128, gmax, upc], bf16, name="lt_tile")
                nc.gpsimd.dma_start(lt[:, 0:gs, :], lt_view[:, c0 : c0 + gs, :])
                # spread the embedding-row gathers through the stream so
                # their descriptor generation hides under the big loads
                if g >= 1:
                    k0 = (g - 1) * len(gather_slices) // (n_groups - 1)
                    k1 = g * len(gather_slices) // (n_groups - 1)
                    for nm, j in gather_slices[k0:k1]:
                        gather_slice(nm, j)
                for j in range(gs):
                    c = c0 + j
                    st = c == 0
                    sp = c == KCHUNKS - 1
                    lhs = t_sb[:, c, :]
                    nc.tensor.matmul(
                        accA[:], lhs, lt[:, j, 0:nA], start=st, stop=sp
                    )
                    if accB is not None:
                        nc.tensor.matmul(
                            accB[:], lhs, lt[:, j, nA:upc], start=st, stop=sp
                        )
                    oh = ohpool.tile([128, CLUSTER], bf16, name="oh")
                    nc.vector.tensor_scalar(
                        oh[:], iota_sb[:], cid_sb[:, c : c + 1], None, EQ
                    )
                    nc.tensor.matmul(
                        accC[:],
                        tcl_sb[:] if sp else lhs,
                        oh[:],
                        start=st,
                        stop=sp,
                    )
                c0 += gs

            # ---- embedding gather writeback ----
            for nm, out in (("uidx", EMB["ue_out"]), ("pidx", EMB["pe_out"]),
                            ("nidx", EMB["ne_out"])):
                nc.scalar.dma_start(
                    out[:].rearrange("(j p) d -> p j d", p=128), g_sb[nm][:]
                )

            # ---- finalize centers (emitted first: longest tail chain) ----
            c_sb = outpool.tile([MAUG, CLUSTER], f32, bufs=1)
            nc.vector.tensor_copy(c_sb[:], accC[:])
            nc.vector.tensor_scalar(
                c_sb[64:65, :], c_sb[64:65, :], 1.0, None, mybir.AluOpType.max
            )
            ce = []
            for h in range(2):
                ctp = tppool.tile([128, MAUG], f32, name=f"ctp{h}", tag="tps")
                nc.tensor.matmul(
                    ctp[:], c_sb[:, h * 128 : (h + 1) * 128], eye_sb[:],
                    is_transpose=True,
                )
                rc = outpool.tile([128, 1], f32, name=f"rc{h}")
                nc.vector.reciprocal(rc[:], ctp[:, 64:65])
                ce_sb = outpool.tile([128, DIM], f16, name=f"ce_sb{h}", bufs=1)
                nc.vector.tensor_scalar(ce_sb[:], ctp[:, 0:DIM], rc[:], None, MUL)
                ce.append(ce_sb)

            # ---- pos/neg centers via onehot matmuls ----
            # onehot[p, b] = (cid[batch b] == h*128 + p); each column has
            # exactly one nonzero, so centers.T @ onehot is a gather (fp16
            # rounds the gathered center values only).
            for rep_sb, out in ((cpr_sb, PCT), (cnr_sb, NCT)):
                pcps = tppool.tile([DIM, nbpc], f32, name="pcps", tag="pcps",
                                   bufs=1)
                for h in range(2):
                    oh2 = ohpool.tile([128, nbpc], f16, name="oh2")
                    nc.vector.tensor_scalar(
                        oh2[:], rep_sb[:], pcol_sb[:, h : h + 1], None, EQ
                    )
                    for q in range(0, nbpc, 512):
                        qe = min(q + 512, nbpc)
                        nc.tensor.matmul(
                            pcps[:, q:qe], ce[h][:], oh2[:, q:qe],
                            start=(h == 0), stop=(h == 1),
                        )
                pct_sb = outpool.tile([DIM, nbpc], f32, name="pct_sb")
                nc.vector.tensor_copy(pct_sb[:], pcps[:])
                nc.scalar.dma_start(out[:], pct_sb[:])

            # ---- finalize uni_center ----
            p_sb = outpool.tile([MAUG, upc], f32, bufs=1)
            nc.vector.tensor_copy(p_sb[:, 0:nA], accA[:])
            if accB is not None:
                nc.vector.tensor_copy(p_sb[:, nA:upc], accB[:])
            for bi, (o, blk) in enumerate(blocks):
                ptp = tppool.tile([128, MAUG], f32, name="ptp", tag="tps")
                nc.tensor.matmul(
                    ptp[0:blk, :], p_sb[:, o : o + blk], eye_sb[:],
                    is_transpose=True,
                )
                r = outpool.tile([128, 1], f32, name="recip")
                nc.vector.reciprocal(r[0:blk, :], ptp[0:blk, 64:65])
                u_sb = outpool.tile([128, DIM], f32, name="u_sb")
                nc.vector.tensor_scalar(
                    u_sb[0:blk, :], ptp[0:blk, 0:DIM], r[0:blk, :], None, MUL
                )
                eng = nc.sync if bi % 2 == 0 else nc.scalar
                eng.dma_start(UNI[o : o + blk, :], u_sb[0:blk, :])

    split_multiwaits(nc)
    return nc


# ------------------------- host side -------------------------

def _wrap_idx(idx: np.ndarray) -> np.ndarray:
    """indirect gather layout: element [p, j] = idx[j*128 + p]."""
    n = idx.shape[0]
    return np.ascontiguousarray(idx.astype(np.int32).reshape(n // 128, 128).T)


def host_prep(user, pos, neg, cluster_ids, user_table, item_table, train_label):
    user = np.asarray(user).astype(np.int64)
    pos = np.asarray(pos).astype(np.int64)
    neg = np.asarray(neg).astype(np.int64)
    cluster_ids = np.asarray(cluster_ids).astype(np.int64)
    user_table = np.ascontiguousarray(np.asarray(user_table, dtype=np.float32))
    item_table = np.ascontiguousarray(np.asarray(item_table, dtype=np.float32))
    train_label = np.asarray(train_label, dtype=np.float32)

    uniq, inverse = np.unique(user, return_inverse=True)
    nu = len(uniq)
    upc = -(-nu // (NCORES * 16)) * 16  # per-core users, mult of 16
    upad = upc * NCORES
    uu = np.concatenate([uniq, np.full(upad - nu, uniq[0], dtype=uniq.dtype)])

    # T_aug partition-major [128, KCHUNKS, MAUG] bf16
    t_aug = np.zeros((KPAD, MAUG), np.float32)
    t_aug[: NUM_ITEMS, :DIM] = item_table[:NUM_ITEMS]  # row 20000 zeroed
    t_aug[: NUM_ITEMS + 1, DIM] = 1.0
    t_pm = np.ascontiguousarray(
        t_aug.reshape(KCHUNKS, 128, MAUG).transpose(1, 0, 2).reshape(128, -1)
    ).astype(ml_dtypes.bfloat16)
    # centers variant of the last chunk: real row 20000
    last = np.zeros((128, MAUG), np.float32)
    lo = (KCHUNKS - 1) * 128
    nreal = NUM_ITEMS + 1 - lo
    last[:nreal, :DIM] = item_table[lo : NUM_ITEMS + 1]
    last[:nreal, DIM] = 1.0
    t_cl = last.astype(ml_dtypes.bfloat16)

    eye66 = np.eye(MAUG, dtype=np.float32)
    iota256 = np.broadcast_to(
        np.arange(CLUSTER, dtype=np.float32), (128, CLUSTER)
    ).copy()
    cid_pm = np.full((KPAD,), -1.0, np.float32)
    cid_pm[: NUM_ITEMS + 1] = cluster_ids.astype(np.float32)
    cid_pm = np.ascontiguousarray(cid_pm.reshape(KCHUNKS, 128).T)
    pcol = (np.arange(128, dtype=np.float32)[:, None]
            + np.array([0.0, 128.0], np.float32)[None, :])
    pcol = np.ascontiguousarray(pcol)

    cpos = cluster_ids[pos].astype(np.float32)
    cneg = cluster_ids[neg].astype(np.float32)

    nbpc = BATCH // NCORES
    shared = {
        "t_pm": t_pm,
        "t_cl": t_cl,
        "eye66": eye66,
        "iota256": iota256,
        "cid_pm": cid_pm,
        "pcol": pcol,
        "user_table": user_table,
        "item_table": item_table,
    }
    in_maps = []
    for c in range(NCORES):
        rows = uu[c * upc : (c + 1) * upc]
        gathered = train_label[rows]  # [upc, 20001]
        lt = np.zeros((KPAD, upc), np.float32)
        lt[: NUM_ITEMS + 1, :] = gathered.T
        # partition-major: ltpm[p, c, u] = lt[c*128 + p, u]
        ltpm = np.ascontiguousarray(
            lt.reshape(KCHUNKS, 128, upc).transpose(1, 0, 2)
        )
        bs = slice(c * nbpc, (c + 1) * nbpc)
        m = dict(shared)
        m["lt"] = ltpm
        m["uidx"] = _wrap_idx(user[bs])
        m["pidx"] = _wrap_idx(pos[bs])
        m["nidx"] = _wrap_idx(neg[bs])
        m["cpr"] = np.ascontiguousarray(
            np.broadcast_to(cpos[bs][None, :], (128, nbpc))
        ).astype(ml_dtypes.bfloat16)
        m["cnr"] = np.ascontiguousarray(
            np.broadcast_to(cneg[bs][None, :], (128, nbpc))
        ).astype(ml_dtypes.bfloat16)
        in_maps.append(m)

    meta = {"upc": upc, "nbpc": nbpc, "nu": nu, "inverse": inverse}
    return in_maps, meta


def assemble(results, meta):
    inverse = meta["inverse"]
    uni_unique = np.concatenate([r["uni_part"] for r in results], axis=0)
    uni = uni_unique[inverse]
    ue = np.concatenate([r["ue_out"] for r in results], axis=0)
    pe = np.concatenate([r["pe_out"] for r in results], axis=0)
    ne = np.concatenate([r["ne_out"] for r in results], axis=0)
    pc = np.concatenate([r["pct_out"].T for r in results], axis=0)
    ncen = np.concatenate([r["nct_out"].T for r in results], axis=0)
    return ue, pe, ne, pc, ncen, uni


_CACHE = {}


def _run(in_maps, meta, trace=False):
    from concourse.bass_utils import run_bass_kernel_spmd

    key = (meta["upc"], meta["nbpc"])
    if key not in _CACHE:
        _CACHE[key] = build_bass(*key)
    nc = _CACHE[key]
    res = run_bass_kernel_spmd(
        nc, in_maps, core_ids=list(range(NCORES)), trace=trace
    )
    return res


def kernel(user, pos, neg, cluster_ids, user_table, item_table, train_label):
    """Full (unsharded) inputs -> full outputs, computed on 8 NeuronCores."""
    in_maps, meta = host_prep(
        user, pos, neg, cluster_ids, user_table, item_table, train_label
    )
    res = _run(in_maps, meta)
    return assemble(res.results, meta)



# revision 10
# speedup vs baseline: 2.9006x; 2.9006x over previous
"""Trainium2 Bass kernel for nn_MatrixFactorization (segment_reduce).

Decomposition (8 cores, SPMD, no collectives):
  - Dedup users of the batch -> unique users, sharded 8 ways (upc per core).
  - Dual-dtype fp8 label packing: each byte holds TWO users' 0/1 labels for
    one item, encoded so that the byte's value under the e4m3 view and under
    the e5m2 view are both affine in (b1, b2):
        byte(b1,b2) in {0x34, 0x40, 0xC0, 0xB4}
        e4m3: 0.75 + 1.25 b1 - 2.75 b2      e5m2: 0.25 + 1.75 b1 - 2.25 b2
    The device matmuls the same SBUF bytes twice (rhs bitcast e4m3 / e5m2,
    DoubleRow perf mode, K=256 per instruction at 0.5 cyc/col) against
    hi/lo-split fp8 item weights, subtracts the all-(0x34) S-column, and
    solves the 2x2 system on DVE:
        P1 = -1.125 A' + 1.375 B'   P2 = -0.875 A' + 0.625 B'
    This halves label HBM traffic vs bf16-per-user and the reconstruction is
    exact in (b1, b2); the only error is the fp8 hi+lo residual of the item
    table (~2^-8 relative).
  - T_pm weights [128, 79dc, 2slot, 144]: cols 0:64 = fp8 hi(item), 64 = ones
    (num_rel), 65:129 = fp8 lo residual, 129 = zero (so the lo pass has the
    same [65, N] psum extent as the hi pass), rest pad (ldweights needs the
    slot pitch to be a multiple of 16 -- 's3_lw_dual_fp8_restrictions').
  - Cluster centers: items are HOST-SORTED by cluster id, so each 256-item
    double chunk touches clusters inside at most two 32-wide windows. DVE
    builds tiny [128, 32] onehot slices (is_equal of iota32 vs cid-32w); PE
    uses them as the *stationary* DoubleRow operand with T_pm slices moving,
    accumulating [32cl, 65] into the 32-row window of the [128, 65] psum of
    the matching cluster half. Sums land pre-transposed; counts ride the
    ones column; centers finish as soon as T_pm is loaded (mid-stream), so
    the pos/neg center outputs fully overlap the label stream.
    The double chunk containing item 20000 uses a T_cl variant with its real
    row (zeroed in T_pm because uni excludes it, but centers include it).
  - pos/neg centers: host-built fp8 onehot slot tensors [128, 2half, nbpc];
    centers re-quantized to fp8 hi+lo; two DoubleRow passes -> [64, nbpc].
  - user/pos/neg embeddings: GPSIMD indirect row gathers, cast to f16.
"""

import numpy as np
import ml_dtypes

import concourse.bass as bass
import concourse.mybir as mybir
import concourse.tile as tile

NUM_USERS = 10000
NUM_ITEMS = 20000
DIM = 64
CLUSTER = 256
BATCH = 8192
NCORES = 8

DC = 79                  # double chunks of K=256
KCH = 2 * DC             # 158 chunks of 128
KPAD = KCH * 128         # 20224
MP = 144                 # T_pm free pitch per (dc, slot): hi64|ones|lo64|zero|pad
GROUPS = [1, 2, 4] + [8] * 9  # double-chunk group sizes (sum = 79)
assert sum(GROUPS) == DC

# dual-view byte quadruple (see module docstring)
B00, B10, B01, B11 = 0x34, 0x40, 0xC0, 0xB4
M11, M12 = -1.125, 1.375   # P1 = M11*A' + M12*B'
M21, M22 = -0.875, 0.625   # P2 = M21*A' + M22*B'


def split_multiwaits(nc):
    """nix-walrus accepts at most ONE sync-wait per instruction; Tile attaches
    many. Hoist all but the last wait onto single-wait NoOps inserted just
    before the instruction, on the same engine."""
    n_split = 0
    for f in nc.m.functions:
        for bb in f.blocks:
            il = list(bb.instructions)
            new = []
            changed = False
            for ins in il:
                si = ins.sync_info
                if si is not None and si.on_wait is not None and len(si.on_wait) > 1:
                    waits = list(si.on_wait)
                    for k, w in enumerate(waits[:-1]):
                        nop = mybir.InstNoOp(
                            name=f"{ins.name}-wsplit{k}", ins=[], outs=[]
                        )
                        nop.engine = ins.engine
                        nop.sync_info = mybir.SyncInfo(on_wait=[w], on_update=[])
                        new.append(nop)
                    ins.sync_info = mybir.SyncInfo(
                        on_wait=waits[-1:], on_update=list(si.on_update or [])
                    )
                    changed = True
                    n_split += 1
                new.append(ins)
            if changed:
                bb.instructions = new
    return n_split


def build_bass(upc: int, nbpc: int, cspec):
    """upc: unique users per core (even); nbpc: batch entries per core.
    cspec: (qdc, cops) where cops[dc] is a tuple of
    (w, col_s0, col_s1, start, stop) center-window ops."""
    f32 = mybir.dt.float32
    bf16 = mybir.dt.bfloat16
    f16 = mybir.dt.float16
    e4 = mybir.dt.float8e4
    e5 = mybir.dt.float8e5
    i32 = mybir.dt.int32
    EQ = mybir.AluOpType.is_equal
    MUL = mybir.AluOpType.mult
    ADD = mybir.AluOpType.add
    SUB = mybir.AluOpType.subtract
    MAX = mybir.AluOpType.max
    DR = mybir.MatmulPerfMode.DoubleRow

    qdc, cops = cspec
    ncw = sum(len(x) for x in cops) * 2  # cidw columns

    assert upc % 2 == 0 and nbpc % 128 == 0
    pairs = upc // 2
    NB = pairs + 1          # +1 S-column of B00 bytes
    jg = nbpc // 128

    nc = bass.Bass(trn_type="TRN2")

    # ---- I/O ----
    LT = nc.dram_tensor("lt", [128, KCH, NB], e4, kind="ExternalInput")
    TPM = nc.dram_tensor("t_pm", [128, DC, 2, MP], e4, kind="ExternalInput")
    TCL = nc.dram_tensor("t_cl", [128, 2, MP], e4, kind="ExternalInput")
    IOTA = nc.dram_tensor("iota64", [128, 64], f32, kind="ExternalInput")
    CIDW = nc.dram_tensor("cidw", [128, ncw], f32, kind="ExternalInput")
    EYE = nc.dram_tensor("eye65", [65, 65], f32, kind="ExternalInput")
    OHP = nc.dram_tensor("ohp", [128, 2, nbpc], e4, kind="ExternalInput")
    OHN = nc.dram_tensor("ohn", [128, 2, nbpc], e4, kind="ExternalInput")
    UT = nc.dram_tensor("user_table", [NUM_USERS, DIM], f32, kind="ExternalInput")
    IT = nc.dram_tensor("item_table", [NUM_ITEMS + 1, DIM], f32, kind="ExternalInput")
    IDX = {}
    for nm in ("uidx", "pidx", "nidx"):
        IDX[nm] = nc.dram_tensor(nm, [128, jg], i32, kind="ExternalInput")

    UNI = nc.dram_tensor("uni_part", [upc, DIM], f32, kind="ExternalOutput")
    EMB = {}
    for nm in ("ue_out", "pe_out", "ne_out"):
        EMB[nm] = nc.dram_tensor(nm, [nbpc, DIM], f16, kind="ExternalOutput")
    PCT = nc.dram_tensor("pct_out", [DIM, nbpc], bf16, kind="ExternalOutput")
    NCT = nc.dram_tensor("nct_out", [DIM, nbpc], bf16, kind="ExternalOutput")

    gmax = max(GROUPS)

    with tile.TileContext(nc) as tc:
        with (
            tc.tile_pool(name="const", bufs=1) as cpool,
            tc.tile_pool(name="ltp", bufs=3) as ltpool,
            tc.tile_pool(name="ohp", bufs=3) as ohpool,
            tc.tile_pool(name="acc", bufs=1, space="PSUM") as accpool,
            tc.tile_pool(name="tp", bufs=2, space="PSUM") as tppool,
            tc.tile_pool(name="pp", bufs=2, space="PSUM") as pppool,
            tc.tile_pool(name="outp", bufs=3) as outpool,
        ):
            # ---- constants into SBUF (scalar/Act ring; labels ride sync) ----
            t_sb = cpool.tile([128, DC, 2, MP], e4)
            iota_sb = cpool.tile([128, 64], f32)
            cidw_sb = cpool.tile([128, ncw], f32)
            nc.scalar.dma_start(t_sb[:, 0:3, :, :], TPM[:, 0:3, :, :])
            nc.scalar.dma_start(iota_sb[:], IOTA[:])
            nc.scalar.dma_start(cidw_sb[:], CIDW[:])
            idx_sb = {}
            g_sb = {}
            for nm, h in IDX.items():
                s = cpool.tile([128, jg], i32, name=f"idx_{nm}")
                nc.scalar.dma_start(s[:], h[:])
                idx_sb[nm] = s
                g_sb[nm] = cpool.tile([128, jg, DIM], f16, name=f"g_{nm}")
            tb = 3
            for piece in (8, 16, 26, DC):
                te = min(tb + piece, DC)
                nc.scalar.dma_start(t_sb[:, tb:te, :, :], TPM[:, tb:te, :, :])
                tb = te
                if tb == DC:
                    break
            tcl_sb = cpool.tile([128, 2, MP], e4)
            nc.scalar.dma_start(tcl_sb[:], TCL[:])
            ohp_sb = cpool.tile([128, 2, nbpc], e4)
            nc.scalar.dma_start(ohp_sb[:], OHP[:])
            ohn_sb = cpool.tile([128, 2, nbpc], e4)
            nc.scalar.dma_start(ohn_sb[:], OHN[:])
            eye_sb = cpool.tile([65, 65], f32)
            nc.scalar.dma_start(eye_sb[:], EYE[:])

            gsrc = {"uidx": UT[:], "pidx": IT[:], "nidx": IT[:]}

            def gather_slice(nm, j):
                nc.gpsimd.indirect_dma_start(
                    out=g_sb[nm][:, j, :],
                    out_offset=None,
                    in_=gsrc[nm],
                    in_offset=bass.IndirectOffsetOnAxis(
                        ap=idx_sb[nm][:, j : j + 1], axis=0
                    ),
                )

            gather_slices = [(nm, j) for nm in ("uidx", "pidx", "nidx")
                             for j in range(jg)]
            NGG = 6  # spread gathers over groups 1..NGG

            ce8 = outpool.tile([128, 2, 2, DIM], e4, bufs=1)
            ce8o = outpool.tile([64, 2, 2, DIM], e4, bufs=1)
            cenf = outpool.tile([64, 2, DIM], f32, bufs=1)

            # ---- psum accumulators ----
            accA = accpool.tile([65, NB], f32)          # e4m3 view sums
            accB = accpool.tile([65, NB], f32)          # e5m2 view sums
            # center sums: even windows (w0, w2) and odd (w1, w3), slot = w//2.
            # separate tiles so temporally-overlapping windows never share a
            # psum zero region; all matmul dst partitions start at 0.
            cenE = accpool.tile([64, 2, 65], f32)
            cenO = accpool.tile([64, 2, 65], f32)

            # ---- center path emitters (run ahead of the label cursor) ----
            def emit_centers(dc):
                oh = ohpool.tile([128, 2, 2, 64], e4, name="oh")
                cwt = tcl_sb[:] if dc == qdc else t_sb[:, dc, :, :]
                for k, (w, c0, c1, wst, wsp) in enumerate(cops[dc]):
                    for s, cc in ((0, c0), (1, c1)):
                        nc.vector.tensor_scalar(
                            oh[:, s, k, :], iota_sb[:],
                            cidw_sb[:, cc : cc + 1], None, EQ,
                        )
                    ct = cenO if (w % 2) else cenE
                    u = w // 2
                    nc.tensor.matmul(
                        ct[0:64, u, 0:65], oh[:, :, k, :],
                        cwt[:, :, 0:65],
                        start=wst, stop=False, perf_mode=DR,
                    )
                    nc.tensor.matmul(
                        ct[0:64, u, 0:65], oh[:, :, k, :],
                        cwt[:, :, 65:130],
                        start=False, stop=wsp, perf_mode=DR,
                    )

            def emit_center_outputs():
                # centers -> fp8 hi/lo -> pos/neg DoubleRow matmuls -> DMA out
                # cenE slots map to ce8 rows 0:64 (halves 0, 1); cenO slots to
                # rows 64:128 via a tiny SBUF->SBUF partition-shift DMA.
                for ct, dst in ((cenE, ce8[0:64, :, :, :]), (cenO, ce8o[:])):
                    for u in range(2):
                        mx = outpool.tile([64, 1], f32, name="mx")
                        nc.vector.tensor_scalar(mx[:], ct[0:64, u, 64:65], 1.0,
                                                None, MAX)
                        rc = outpool.tile([64, 1], f32, name="rc")
                        nc.vector.reciprocal(rc[:], mx[:])
                        nc.vector.tensor_scalar(
                            cenf[:, u, :], ct[0:64, u, 0:64], rc[:], None, MUL
                        )
                        nc.vector.tensor_copy(dst[:, u, 0, :], cenf[:, u, :])
                        nc.vector.tensor_tensor(
                            dst[:, u, 1, :], cenf[:, u, :], dst[:, u, 0, :], SUB
                        )
                nc.scalar.dma_start(ce8[64:128, :, :, :], ce8o[:])
                for oh_sb, out in ((ohp_sb, PCT), (ohn_sb, NCT)):
                    pct_sb = outpool.tile([DIM, nbpc], bf16, name="pct_sb")
                    for q in range(0, nbpc, 512):
                        pp = pppool.tile([DIM, 512], f32, name="pp", tag="pp")
                        nc.tensor.matmul(pp[:], ce8[:, :, 0, :],
                                         oh_sb[:, :, q : q + 512],
                                         start=True, stop=False, perf_mode=DR)
                        nc.tensor.matmul(pp[:], ce8[:, :, 1, :],
                                         oh_sb[:, :, q : q + 512],
                                         start=False, stop=True, perf_mode=DR)
                        nc.scalar.copy(pct_sb[:, q : q + 512], pp[:])
                    nc.scalar.dma_start(out[:], pct_sb[:])

            # ---- main streaming loop over double chunks ----
            d0 = 0
            cen_done = 0
            for g, gs in enumerate(GROUPS):
                lt = ltpool.tile([128, 2 * gmax, NB], e4, name="lt_tile")
                nc.sync.dma_start(
                    lt[:, 0 : 2 * gs, :], LT[:, 2 * d0 : 2 * (d0 + gs), :]
                )
                if 1 <= g <= NGG:
                    k0 = (g - 1) * len(gather_slices) // NGG
                    k1 = g * len(gather_slices) // NGG
                    for nm, j in gather_slices[k0:k1]:
                        gather_slice(nm, j)
                # centers run ~2x ahead of the label cursor
                ctarget = min(DC, 2 * (d0 + gs))
                while cen_done < ctarget:
                    emit_centers(cen_done)
                    cen_done += 1
                    if cen_done == DC:
                        emit_center_outputs()
                for j in range(gs):
                    dc = d0 + j
                    st = dc == 0
                    sp = dc == DC - 1
                    rhs = lt[:, 2 * j : 2 * j + 2, :]
                    wt = t_sb[:, dc, :, :]
                    nc.tensor.matmul(accA[:], wt[:, :, 0:65], rhs,
                                     start=st, stop=False, perf_mode=DR)
                    nc.tensor.matmul(accA[:], wt[:, :, 65:130], rhs,
                                     start=False, stop=sp, perf_mode=DR)
                    rhs5 = rhs.bitcast(e5)
                    nc.tensor.matmul(accB[:], wt[:, :, 0:65], rhs5,
                                     start=st, stop=False, perf_mode=DR)
                    nc.tensor.matmul(accB[:], wt[:, :, 65:130], rhs5,
                                     start=False, stop=sp, perf_mode=DR)
                d0 += gs
                if g == NGG:
                    # gathers done: write embeddings out mid-stream
                    for nm, out in (("uidx", EMB["ue_out"]),
                                    ("pidx", EMB["pe_out"]),
                                    ("nidx", EMB["ne_out"])):
                        nc.scalar.dma_start(
                            out[:].rearrange("(j p) d -> p j d", p=128),
                            g_sb[nm][:],
                        )

            # ---- uni: solve the 2x2 view system, transpose, divide ----
            sc = outpool.tile([65, 2], f32, bufs=1)
            tmp_s = outpool.tile([65, 2], f32, bufs=1)
            for i, (ma, mb_) in enumerate(((M11, M12), (M21, M22))):
                nc.vector.tensor_scalar(
                    tmp_s[:, i : i + 1], accB[:, pairs : pairs + 1], mb_, None, MUL
                )
                nc.vector.scalar_tensor_tensor(
                    sc[:, i : i + 1], accA[:, pairs : pairs + 1], ma,
                    tmp_s[:, i : i + 1], MUL, ADD,
                )
            p_sb = outpool.tile([65, upc], f32, bufs=1)
            tmp_b = outpool.tile([65, pairs], f32, bufs=1)
            # recombine in column chunks so transposes pipeline behind DVE
            h1 = (pairs + 1) // 2
            chunks = [(0, 0, h1), (0, h1, pairs), (1, 0, h1), (1, h1, pairs)]
            for i, lo_, hi_ in chunks:
                ma, mb_ = (M11, M12) if i == 0 else (M21, M22)
                nc.vector.tensor_scalar(
                    tmp_b[:, lo_:hi_], accB[:, lo_:hi_], mb_,
                    sc[:, i : i + 1], MUL, SUB,
                )
                nc.vector.scalar_tensor_tensor(
                    p_sb[:, i * pairs + lo_ : i * pairs + hi_],
                    accA[:, lo_:hi_], ma, tmp_b[:, lo_:hi_], MUL, ADD,
                )
            o = 0
            bi = 0
            while o < upc:
                blk = min(128, upc - o)
                ptp = tppool.tile([128, 65], f32, name="ptp", tag="tps")
                nc.tensor.matmul(
                    ptp[0:blk, :], p_sb[:, o : o + blk], eye_sb[:],
                    is_transpose=True,
                )
                r = outpool.tile([128, 1], f32, name="recip")
                nc.vector.reciprocal(r[0:blk, :], ptp[0:blk, 64:65])
                u_sb = outpool.tile([128, DIM], f32, name="u_sb")
                nc.vector.tensor_scalar(
                    u_sb[0:blk, :], ptp[0:blk, 0:DIM], r[0:blk, :], None, MUL
                )
                eng = nc.sync if bi % 2 == 0 else nc.scalar
                eng.dma_start(UNI[o : o + blk, :], u_sb[0:blk, :])
                o += blk
                bi += 1

    split_multiwaits(nc)
    return nc


# ------------------------- host side -------------------------

def _wrap_idx(idx: np.ndarray) -> np.ndarray:
    """indirect gather layout: element [p, j] = idx[j*128 + p]."""
    n = idx.shape[0]
    return np.ascontiguousarray(idx.astype(np.int32).reshape(n // 128, 128).T)


def _fp8(x):
    return x.astype(ml_dtypes.float8_e4m3)


def host_prep(user, pos, neg, cluster_ids, user_table, item_table, train_label):
    user = np.asarray(user).astype(np.int64)
    pos = np.asarray(pos).astype(np.int64)
    neg = np.asarray(neg).astype(np.int64)
    cluster_ids = np.asarray(cluster_ids).astype(np.int64)
    user_table = np.ascontiguousarray(np.asarray(user_table, dtype=np.float32))
    item_table = np.ascontiguousarray(np.asarray(item_table, dtype=np.float32))
    train_label = np.asarray(train_label, dtype=np.float32)

    uniq, inverse = np.unique(user, return_inverse=True)
    nu = len(uniq)
    upc = 2 * (-(-nu // (NCORES * 2)))  # per-core users, even
    upad = upc * NCORES
    uu = np.concatenate([uniq, np.full(upad - nu, uniq[0], dtype=uniq.dtype)])
    pairs = upc // 2
    NB = pairs + 1

    # ---- sort items by cluster id ----
    order = np.argsort(cluster_ids, kind="stable")     # [20001]
    cs = cluster_ids[order]                            # sorted cids
    q = int(np.nonzero(order == NUM_ITEMS)[0][0])      # sorted pos of item 20000
    qdc = q // 256

    # ---- T_pm: [128, DC, 2, MP] fp8e4; hi|ones|lo|zero|pad, row q zeroed
    t_sorted = item_table[order]
    t_uni = t_sorted.copy()
    t_uni[q] = 0.0
    hi = _fp8(t_uni)
    lo = _fp8(t_uni - hi.astype(np.float32))
    taug = np.zeros((KPAD, MP), ml_dtypes.float8_e4m3)
    taug[: NUM_ITEMS + 1, 0:DIM] = hi
    taug[: NUM_ITEMS + 1, DIM] = 1.0
    taug[: NUM_ITEMS + 1, 65 : 65 + DIM] = lo
    t_pm = np.ascontiguousarray(
        taug.reshape(DC, 2, 128, MP).transpose(2, 0, 1, 3)
    )
    # centers variant of double chunk qdc: real row q
    hic = _fp8(item_table[NUM_ITEMS])
    loc = _fp8(item_table[NUM_ITEMS] - hic.astype(np.float32))
    tcl = taug[qdc * 256 : (qdc + 1) * 256].copy()
    tcl[q - qdc * 256, 0:DIM] = hic
    tcl[q - qdc * 256, 65 : 65 + DIM] = loc
    t_cl = np.ascontiguousarray(tcl.reshape(2, 128, MP).transpose(1, 0, 2))

    # ---- center window schedule + cidw columns ----
    iota64 = np.broadcast_to(
        np.arange(64, dtype=np.float32), (128, 64)
    ).copy()
    BIG = 10 ** 6
    cs_pad = np.full(KPAD, -BIG, np.int64)
    cs_pad[: NUM_ITEMS + 1] = cs
    chunk_wins = []
    for ch in range(KCH):
        seg = cs_pad[128 * ch : 128 * ch + 128]
        real = seg[seg >= 0]
        if len(real) == 0:
            chunk_wins.append(())
        else:
            chunk_wins.append(tuple(range(int(real[0]) >> 6,
                                          (int(real[-1]) >> 6) + 1)))
    first_dc = {}
    last_dc = {}
    for dc in range(DC):
        ws = set(chunk_wins[2 * dc]) | set(chunk_wins[2 * dc + 1])
        for w in ws:
            first_dc.setdefault(w, dc)
            last_dc[w] = dc
    cidw_cols = []
    cops = []
    for dc in range(DC):
        ws = sorted(set(chunk_wins[2 * dc]) | set(chunk_wins[2 * dc + 1]))
        assert len(ws) <= 2, (dc, ws)
        ent = []
        for w in ws:
            cc = []
            for s in (0, 1):
                col = cs_pad[128 * (2 * dc + s) : 128 * (2 * dc + s) + 128]
                colv = np.where(col >= 0, col - 64 * w, -BIG).astype(np.float32)
                cc.append(len(cidw_cols))
                cidw_cols.append(colv)
            ent.append((w, cc[0], cc[1], first_dc[w] == dc, last_dc[w] == dc))
        cops.append(tuple(ent))
    cidw = np.ascontiguousarray(np.stack(cidw_cols, axis=1))
    cspec = (qdc, tuple(cops))

    eye65 = np.eye(65, dtype=np.float32)

    # pos/neg onehot slot tensors [128, 2, nbpc] fp8 (exact 0/1)
    nbpc = BATCH // NCORES
    cpos = cluster_ids[pos].astype(np.int64)
    cneg = cluster_ids[neg].astype(np.int64)

    def oh_slots(cids):
        oh = np.zeros((128, 2, len(cids)), np.uint8)
        half = (cids >= 128).astype(np.int64)
        oh[cids % 128, half, np.arange(len(cids))] = 0x38  # fp8e4m3 1.0
        return oh.view(ml_dtypes.float8_e4m3)

    # label byte LUT: index b1 + 2*b2
    lut = np.array([B00, B10, B01, B11], np.uint8)

    shared = {
        "t_pm": t_pm,
        "t_cl": t_cl,
        "iota64": iota64,
        "cidw": cidw,
        "eye65": eye65,
        "user_table": user_table,
        "item_table": item_table,
    }
    in_maps = []
    for c in range(NCORES):
        rows = uu[c * upc : (c + 1) * upc]
        bits = (train_label[rows][:, order] > 0.5).astype(np.uint8)
        code = bits[0:pairs] + 2 * bits[pairs : 2 * pairs]  # [pairs, 20001]
        lt = np.full((KPAD, NB), B00, np.uint8)
        lt[: NUM_ITEMS + 1, 0:pairs] = lut[code].T
        ltpm = np.ascontiguousarray(
            lt.reshape(KCH, 128, NB).transpose(1, 0, 2)
        ).view(ml_dtypes.float8_e4m3)
        bs = slice(c * nbpc, (c + 1) * nbpc)
        m = dict(shared)
        m["lt"] = ltpm
        m["uidx"] = _wrap_idx(user[bs])
        m["pidx"] = _wrap_idx(pos[bs])
        m["nidx"] = _wrap_idx(neg[bs])
        m["ohp"] = np.ascontiguousarray(oh_slots(cpos[bs]))
        m["ohn"] = np.ascontiguousarray(oh_slots(cneg[bs]))
        in_maps.append(m)

    meta = {"upc": upc, "nbpc": nbpc, "nu": nu, "inverse": inverse,
            "cspec": cspec}
    return in_maps, meta


def assemble(results, meta):
    inverse = meta["inverse"]
    uni_unique = np.concatenate([r["uni_part"] for r in results], axis=0)
    uni = uni_unique[inverse]
    ue = np.concatenate(
        [np.asarray(r["ue_out"]).astype(np.float32) for r in results], axis=0
    )
    pe = np.concatenate(
        [np.asarray(r["pe_out"]).astype(np.float32) for r in results], axis=0
    )
    ne = np.concatenate(
        [np.asarray(r["ne_out"]).astype(np.float32) for r in results], axis=0
    )
    pc = np.concatenate(
        [np.asarray(r["pct_out"]).astype(np.float32).T for r in results], axis=0
    )
    ncen = np.concatenate(
        [np.asarray(r["nct_out"]).astype(np.float32).T for r in results], axis=0
    )
    return ue, pe, ne, pc, ncen, uni


_CACHE = {}


def _run(in_maps, meta, trace=False):
    from concourse.bass_utils import run_bass_kernel_spmd

    key = (meta["upc"], meta["nbpc"], meta["cspec"])
    if key not in _CACHE:
        _CACHE[key] = build_bass(meta["upc"], meta["nbpc"], meta["cspec"])
    nc = _CACHE[key]
    res = run_bass_kernel_spmd(
        nc, in_maps, core_ids=list(range(NCORES)), trace=trace
    )
    return res


def kernel(user, pos, neg, cluster_ids, user_table, item_table, train_label):
    """Full (unsharded) inputs -> full outputs, computed on 8 NeuronCores."""
    in_maps, meta = host_prep(
        user, pos, neg, cluster_ids, user_table, item_table, train_label
    )
    res = _run(in_maps, meta)
    return assemble(res.results, meta)
